# revision 4
# baseline (speedup 1.0000x reference)
"""Fused multi-head attention block (qkv proj + attention + out proj) for
Trainium2, batch-parallel across 8 NeuronCores.

Problem shapes (hardcoded): x [8, 1024, 768], w_qkv [2304, 768],
w_proj [768, 768], b_proj [768]; H=12 heads, HD=64.

Each core processes one batch element b:
  phase 1: qkT = w_qk @ x_b.T  -> q,k in transposed layout [hd, npos] (fp32r)
           v   = x_b @ w_v.T   -> natural layout [npos, hd] in bf16, stored
                 with per-head ones-columns appended for softmax sums
  phase 2: per head: S.T = k.T.T @ q.T (fp32r, K=64 row-tiled head pairs),
           P.T = exp(S.T / 8) on ACT (bf16; max-subtraction skipped -- scores
           are ~N(0,1), max ~5.5, exp stays < 300), then
           [av; sums].T = [V|1].T @ P.T (bf16), normalized by broadcasting
           1/sums across partitions; result written as attn.T (fp32r)
  phase 3: out = attn.T.T @ w_proj.T + b_proj (fp32r) -> [npos, 768]
"""
import numpy as np

import concourse.bacc as bacc
import concourse.tile as tile
from concourse import mybir
from concourse.bass_utils import run_bass_kernel_spmd

B, N, C = 8, 1024, 768
H, HD = 12, 64
P = 128
NCORES = 8
F32 = mybir.dt.float32
F32R = mybir.dt.float32r
BF16 = mybir.dt.bfloat16
Exp = mybir.ActivationFunctionType.Exp

KC = C // P          # 6 contraction chunks of 128 over C
NT = N // P          # 8 npos tiles of 128
QC = 2               # qpos halves of 512
NPAIR = H // 2       # 6 head pairs


def build_nc():
    nc = bacc.Bacc("TRN2", target_bir_lowering=False, debug=False)

    xt = nc.declare_dram_parameter("xt", [C, N], F32R, isOutput=False)
    wqk = nc.declare_dram_parameter("wqk", [C, 2 * C], F32R, isOutput=False)
    wv = nc.declare_dram_parameter("wv", [C, C], F32R, isOutput=False)
    wproj = nc.declare_dram_parameter("wproj", [C, C], F32R, isOutput=False)
    bias = nc.declare_dram_parameter("bias", [P, C], F32, isOutput=False)
    out = nc.declare_dram_parameter("out", [N, C], F32, isOutput=True)

    with tile.TileContext(nc) as tc:
        with tc.tile_pool(name="qk", bufs=1) as qk_pool, \
             tc.tile_pool(name="vsb", bufs=1) as v_pool, \
             tc.tile_pool(name="attnT", bufs=1) as at_pool:

            # persistent SBUF tensors
            qk_sb = [qk_pool.tile([P, N], F32R, tag=f"qk{i}", name=f"qk{i}") for i in range(12)]
            # v + ones column per head: [128, 12 heads, 65]; head h cols:
            #   0..63 = V, 64 = ones (for softmax sums)
            v_sb = [v_pool.tile([P, H, 65], BF16, tag=f"v{i}", name=f"v{i}") for i in range(NT)]
            attnT = [at_pool.tile([P, N], F32R, tag=f"at{i}", name=f"at{i}") for i in range(NPAIR)]

            # ---------------- phase 1: qkv projections ----------------
            with tc.tile_pool(name="p1in", bufs=1) as p1in, \
                 tc.tile_pool(name="p1ps", bufs=4, space="PSUM") as p1ps:
                xt_sb = [p1in.tile([P, N], F32R, tag=f"xt{k}", name=f"xts{k}") for k in range(KC)]
                wqk_sb = [p1in.tile([P, 2 * C], F32R, tag=f"wqk{k}", name=f"wqks{k}") for k in range(KC)]
                wv_sb = [p1in.tile([P, C], F32R, tag=f"wv{k}", name=f"wvs{k}") for k in range(KC)]
                for k in range(KC):
                    nc.sync.dma_start(out=xt_sb[k][:], in_=xt[k * P:(k + 1) * P, :])
                    nc.sync.dma_start(out=wqk_sb[k][:], in_=wqk[k * P:(k + 1) * P, :])
                    nc.sync.dma_start(out=wv_sb[k][:], in_=wv[k * P:(k + 1) * P, :])

                # qkT [1536, 1024]: M-tile mt covers rows mt*128..+127
                for mt in range(12):
                    for nh in range(QC):
                        ps = p1ps.tile([P, 512], F32, tag="ps")
                        for k in range(KC):
                            nc.tensor.matmul(
                                ps[:],
                                wqk_sb[k][:, mt * P:(mt + 1) * P],
                                xt_sb[k][:, nh * 512:(nh + 1) * 512],
                                start=(k == 0), stop=(k == KC - 1),
                            )
                        nc.vector.tensor_copy(
                            qk_sb[mt][:, nh * 512:(nh + 1) * 512], ps[:])

                # v natural [1024, 768] -> v_sb bf16 with parity column layout
                for nt in range(NT):
                    nc.vector.memset(v_sb[nt][:, :, 64:65], 1.0)
                    for c0, cw in ((0, 512), (512, 256)):
                        ps = p1ps.tile([P, 512], F32, tag="ps")
                        for k in range(KC):
                            nc.tensor.matmul(
                                ps[:, :cw],
                                xt_sb[k][:, nt * P:(nt + 1) * P],
                                wv_sb[k][:, c0:c0 + cw],
                                start=(k == 0), stop=(k == KC - 1),
                            )
                        nheads_c = cw // 64  # heads in this chunk
                        h_base = c0 // 64
                        psv = ps[:, :cw].rearrange("p (j q) -> p j q", q=64)
                        nc.vector.tensor_copy(
                            v_sb[nt][:, h_base:h_base + nheads_c, 0:64], psv[:])

            # ---------------- phase 2: attention ----------------
            with tc.tile_pool(name="scps", bufs=2, space="PSUM") as sc_ps, \
                 tc.tile_pool(name="avps", bufs=3, space="PSUM") as av_ps_pool, \
                 tc.tile_pool(name="es", bufs=10) as es_pool, \
                 tc.tile_pool(name="rr", bufs=4) as r_pool, \
                 tc.tile_pool(name="p3in", bufs=1) as p3in, \
                 tc.tile_pool(name="osb", bufs=3) as o_pool:

                wproj_sb = [p3in.tile([P, C], F32R, tag=f"wp{k}", name=f"wps{k}") for k in range(KC)]
                bias_sb = p3in.tile([P, C], F32, tag="bias")
                for k in range(KC):
                    nc.sync.dma_start(out=wproj_sb[k][:], in_=wproj[k * P:(k + 1) * P, :])
                nc.sync.dma_start(out=bias_sb[:], in_=bias[:, :])

                def emit_av(p, qc, es_tiles):
                    for par in range(2):
                        h = 2 * p + par
                        av = av_ps_pool.tile([P, 512], F32, tag="av")
                        for kt in range(NT):
                            nc.tensor.matmul(
                                av[0:65, :],
                                v_sb[kt][:, h, :],
                                es_tiles[kt][:, par * 512:(par + 1) * 512],
                                start=(kt == 0), stop=(kt == NT - 1),
                            )
                        rrow = r_pool.tile([P, 512], F32, tag="rrow")
                        # 1-channel DVE op: PSUM part 64 -> SBUF part 0
                        nc.vector.reciprocal(rrow[0:1, :], av[64:65, :])
                        rbc = r_pool.tile([P, 512], F32, tag="rbc")
                        nc.gpsimd.partition_broadcast(rbc[0:64, :], rrow[0:1, :])
                        # 64-channel DVE op: reads parts 0..63, writes the
                        # head's attnT quadrant (0..63 or 64..127)
                        nc.vector.tensor_mul(
                            attnT[p][par * 64:(par + 1) * 64, qc * 512:(qc + 1) * 512],
                            av[0:64, :],
                            rbc[0:64, :])

                def emit_proj(qc):
                    for nt in range(qc * 4, qc * 4 + 4):
                        o_sb = o_pool.tile([P, C], F32, tag="o")
                        for c0, cw in ((0, 512), (512, 256)):
                            ps = av_ps_pool.tile([P, 512], F32, tag="av")
                            for k in range(KC):
                                nc.tensor.matmul(
                                    ps[:, :cw],
                                    attnT[k][:, nt * P:(nt + 1) * P],
                                    wproj_sb[k][:, c0:c0 + cw],
                                    start=(k == 0), stop=(k == KC - 1),
                                )
                            nc.vector.tensor_add(
                                o_sb[:, c0:c0 + cw], ps[:, :cw], bias_sb[:, c0:c0 + cw])
                        nc.sync.dma_start(out=out[nt * P:(nt + 1) * P, :], in_=o_sb[:])

                for qc in range(QC):
                    pending = None
                    for p in range(NPAIR):
                        hA, hB = 2 * p, 2 * p + 1
                        # q head h: qk_sb tile h//2, partitions (h%2)*64..+64
                        # k head h: qk_sb tile 6 + h//2, same partition split
                        qa = qk_sb[p][0:64, qc * 512:(qc + 1) * 512]
                        qb = qk_sb[p][64:128, qc * 512:(qc + 1) * 512]
                        ka = qk_sb[6 + p]
                        kb = qk_sb[6 + p]
                        es_tiles = []
                        for kt in range(NT):
                            ps = sc_ps.tile([P, N], F32, tag="sc")
                            nc.tensor.matmul(
                                ps[:, 0:512],
                                ka[0:64, kt * P:(kt + 1) * P], qa,
                                start=True, stop=True, tile_position=(0, 0),
                            )
                            nc.tensor.matmul(
                                ps[:, 512:1024],
                                kb[64:128, kt * P:(kt + 1) * P], qb,
                                start=True, stop=True, tile_position=(64, 0),
                            )
                            es = es_pool.tile([P, N], BF16, tag="es")
                            nc.scalar.activation(es[:], ps[:], Exp, scale=float(HD) ** -0.5)
                            es_tiles.append(es)
                        if pending is not None:
                            emit_av(*pending)
                        pending = (p, qc, es_tiles)
                    emit_av(*pending)
                    emit_proj(qc)

    nc.finalize()
    return nc


_NC_CACHE = None


def _get_nc():
    global _NC_CACHE
    if _NC_CACHE is None:
        _NC_CACHE = build_nc()
    return _NC_CACHE


def prep_inputs(x, w_qkv, w_proj, b_proj):
    x = np.asarray(x, dtype=np.float32)
    w_qkv = np.asarray(w_qkv, dtype=np.float32)
    w_proj = np.asarray(w_proj, dtype=np.float32)
    b_proj = np.asarray(b_proj, dtype=np.float32)
    wqk = np.ascontiguousarray(w_qkv[:2 * C].T)          # [768, 1536]
    wv = np.ascontiguousarray(w_qkv[2 * C:].T)           # [768, 768]
    wp = np.ascontiguousarray(w_proj.T)                  # [768, 768]
    bias = np.ascontiguousarray(np.tile(b_proj[None, :], (P, 1)))  # [128, 768]
    in_maps = []
    for b in range(NCORES):
        in_maps.append({
            "xt": np.ascontiguousarray(x[b].T),          # [768, 1024]
            "wqk": wqk, "wv": wv, "wproj": wp, "bias": bias,
        })
    return in_maps


def run(in_maps, **kw):
    nc = _get_nc()
    return run_bass_kernel_spmd(nc, in_maps, list(range(NCORES)), **kw)


def kernel(x, w_qkv, w_proj, b_proj):
    res = run(prep_inputs(x, w_qkv, w_proj, b_proj))
    return np.stack([res.results[b]["out"] for b in range(NCORES)], axis=0)


# revision 7
# speedup vs baseline: 1.3678x; 1.3678x over previous
"""Fused multi-head attention block (qkv proj + attention + out proj) for
Trainium2, batch-parallel across 8 NeuronCores.

Problem shapes (hardcoded): x [8, 1024, 768], w_qkv [2304, 768],
w_proj [768, 768], b_proj [768]; H=12 heads, HD=64.

Each core processes one batch element b:
  phase 1: qkT = w_qk @ x_b.T  -> q,k in transposed layout [hd, npos] (fp32r)
           v   = x_b @ w_v.T   -> natural layout [npos, hd] in bf16, stored
                 with per-head ones-columns appended for softmax sums
  phase 2: per head: S.T = k.T.T @ q.T (fp32r, K=64 row-tiled head pairs),
           P.T = exp(S.T / 8) on ACT (bf16; max-subtraction skipped -- scores
           are ~N(0,1), max ~5.5, exp stays < 300), then
           [av; sums].T = [V|1].T @ P.T (bf16), normalized by broadcasting
           1/sums across partitions; result written as attn.T (fp32r)
  phase 3: out = attn.T.T @ w_proj.T + b_proj (fp32r) -> [npos, 768]
"""
import numpy as np

import concourse.bacc as bacc
import concourse.tile as tile
from concourse import mybir
from concourse.bass_utils import run_bass_kernel_spmd

B, N, C = 8, 1024, 768
H, HD = 12, 64
P = 128
NCORES = 8
F32 = mybir.dt.float32
F32R = mybir.dt.float32r
BF16 = mybir.dt.bfloat16
Exp = mybir.ActivationFunctionType.Exp

KC = C // P          # 6 contraction chunks of 128 over C
NT = N // P          # 8 npos tiles of 128
QC = 2               # qpos halves of 512
NPAIR = H // 2       # 6 head pairs


def build_nc():
    nc = bacc.Bacc("TRN2", target_bir_lowering=False, debug=False)

    xt = nc.declare_dram_parameter("xt", [C, N], F32R, isOutput=False)
    wqk = nc.declare_dram_parameter("wqk", [C, 2 * C], F32R, isOutput=False)
    wv = nc.declare_dram_parameter("wv", [C, C], F32R, isOutput=False)
    wproj = nc.declare_dram_parameter("wproj", [C, C], F32R, isOutput=False)
    bias = nc.declare_dram_parameter("bias", [P, C], F32, isOutput=False)
    out = nc.declare_dram_parameter("out", [N, C], F32, isOutput=True)

    with tile.TileContext(nc) as tc:
        with tc.tile_pool(name="qk", bufs=1) as qk_pool, \
             tc.tile_pool(name="vsb", bufs=1) as v_pool, \
             tc.tile_pool(name="attnT", bufs=1) as at_pool:

            # persistent SBUF tensors
            qk_sb = [qk_pool.tile([P, N], F32R, tag=f"qk{i}", name=f"qk{i}") for i in range(12)]
            # v + ones column per head: [128, 12 heads, 65]; head h cols:
            #   0..63 = V, 64 = ones (for softmax sums)
            v_sb = [v_pool.tile([P, H, 65], BF16, tag=f"v{i}", name=f"v{i}") for i in range(NT)]
            attnT = [at_pool.tile([P, N], F32R, tag=f"at{i}", name=f"at{i}") for i in range(NPAIR)]

            # ---------------- phase 1: qkv projections ----------------
            with tc.tile_pool(name="p1in", bufs=1) as p1in, \
                 tc.tile_pool(name="p1ps", bufs=4, space="PSUM") as p1ps:
                xt_sb = [p1in.tile([P, N], F32R, tag=f"xt{k}", name=f"xts{k}") for k in range(KC)]
                wqk_sb = [p1in.tile([P, 2 * C], F32R, tag=f"wqk{k}", name=f"wqks{k}") for k in range(KC)]
                wv_sb = [p1in.tile([P, C], F32R, tag=f"wv{k}", name=f"wvs{k}") for k in range(KC)]
                for k in range(KC):
                    nc.sync.dma_start(out=xt_sb[k][:], in_=xt[k * P:(k + 1) * P, :])
                    nc.sync.dma_start(out=wqk_sb[k][:], in_=wqk[k * P:(k + 1) * P, :])
                    nc.sync.dma_start(out=wv_sb[k][:], in_=wv[k * P:(k + 1) * P, :])

                # qkT [1536, 1024]: M-tile mt covers rows mt*128..+127
                for mt in range(12):
                    for nh in range(QC):
                        ps = p1ps.tile([P, 512], F32, tag="ps")
                        for k in range(KC):
                            nc.tensor.matmul(
                                ps[:],
                                wqk_sb[k][:, mt * P:(mt + 1) * P],
                                xt_sb[k][:, nh * 512:(nh + 1) * 512],
                                start=(k == 0), stop=(k == KC - 1),
                            )
                        nc.vector.tensor_copy(
                            qk_sb[mt][:, nh * 512:(nh + 1) * 512], ps[:])

                # v natural [1024, 768] -> v_sb bf16 with parity column layout
                for nt in range(NT):
                    nc.vector.memset(v_sb[nt][:, :, 64:65], 1.0)
                    for c0, cw in ((0, 512), (512, 256)):
                        ps = p1ps.tile([P, 512], F32, tag="ps")
                        for k in range(KC):
                            nc.tensor.matmul(
                                ps[:, :cw],
                                xt_sb[k][:, nt * P:(nt + 1) * P],
                                wv_sb[k][:, c0:c0 + cw],
                                start=(k == 0), stop=(k == KC - 1),
                            )
                        nheads_c = cw // 64  # heads in this chunk
                        h_base = c0 // 64
                        psv = ps[:, :cw].rearrange("p (j q) -> p j q", q=64)
                        nc.vector.tensor_copy(
                            v_sb[nt][:, h_base:h_base + nheads_c, 0:64], psv[:])

            # ---------------- phase 2: attention ----------------
            with tc.tile_pool(name="scps", bufs=2, space="PSUM") as sc_ps, \
                 tc.tile_pool(name="avps", bufs=3, space="PSUM") as av_ps_pool, \
                 tc.tile_pool(name="es", bufs=10) as es_pool, \
                 tc.tile_pool(name="rr", bufs=6) as r_pool, \
                 tc.tile_pool(name="p3in", bufs=1) as p3in, \
                 tc.tile_pool(name="osb", bufs=3) as o_pool:

                wproj_sb = [p3in.tile([P, C], F32R, tag=f"wp{k}", name=f"wps{k}") for k in range(KC)]
                bias_sb = p3in.tile([P, C], F32, tag="bias")
                for k in range(KC):
                    nc.sync.dma_start(out=wproj_sb[k][:], in_=wproj[k * P:(k + 1) * P, :])
                nc.sync.dma_start(out=bias_sb[:], in_=bias[:, :])

                def emit_av(p, qc, es_tiles):
                    for par in range(2):
                        h = 2 * p + par
                        av = av_ps_pool.tile([P, 512], F32, tag="av")
                        for kt in range(NT):
                            nc.tensor.matmul(
                                av[0:65, :],
                                v_sb[kt][:, h, :],
                                es_tiles[kt][:, par * 512:(par + 1) * 512],
                                start=(kt == 0), stop=(kt == NT - 1),
                            )
                        # evict PSUM immediately so the psum slot frees
                        # without waiting on the normalization chain
                        av_sb = r_pool.tile([P, 512], F32, tag="avsb")
                        nc.vector.tensor_copy(av_sb[0:65, :], av[0:65, :])
                        # stock DVE op: part 64 -> part 0 (cross-quadrant ok)
                        rrow = r_pool.tile([P, 512], F32, tag="rrow")
                        nc.vector.tensor_copy(rrow[0:1, :], av_sb[64:65, :])
                        sbc = r_pool.tile([P, 512], F32, tag="sbc")
                        nc.gpsimd.partition_broadcast(sbc[0:64, :], rrow[0:1, :])
                        rbc = r_pool.tile([P, 512], F32, tag="rbc")
                        # custom-DVE op: base partition 0 only
                        nc.vector.reciprocal_approx_fast(rbc[0:64, :], sbc[0:64, :])
                        # 64-channel DVE op: reads parts 0..63, writes the
                        # head's attnT quadrant (0..63 or 64..127)
                        nc.vector.tensor_mul(
                            attnT[p][par * 64:(par + 1) * 64, qc * 512:(qc + 1) * 512],
                            av_sb[0:64, :],
                            rbc[0:64, :])

                def emit_proj(qc):
                    for nt in range(qc * 4, qc * 4 + 4):
                        o_sb = o_pool.tile([P, C], F32, tag="o")
                        for c0, cw in ((0, 512), (512, 256)):
                            ps = av_ps_pool.tile([P, 512], F32, tag="av")
                            for k in range(KC):
                                nc.tensor.matmul(
                                    ps[:, :cw],
                                    attnT[k][:, nt * P:(nt + 1) * P],
                                    wproj_sb[k][:, c0:c0 + cw],
                                    start=(k == 0), stop=(k == KC - 1),
                                )
                            nc.vector.tensor_add(
                                o_sb[:, c0:c0 + cw], ps[:, :cw], bias_sb[:, c0:c0 + cw])
                        nc.sync.dma_start(out=out[nt * P:(nt + 1) * P, :], in_=o_sb[:])

                for qc in range(QC):
                    pending = None
                    for p in range(NPAIR):
                        hA, hB = 2 * p, 2 * p + 1
                        # q head h: qk_sb tile h//2, partitions (h%2)*64..+64
                        # k head h: qk_sb tile 6 + h//2, same partition split
                        qa = qk_sb[p][0:64, qc * 512:(qc + 1) * 512]
                        qb = qk_sb[p][64:128, qc * 512:(qc + 1) * 512]
                        ka = qk_sb[6 + p]
                        kb = qk_sb[6 + p]
                        es_tiles = []
                        for kt in range(NT):
                            ps = sc_ps.tile([P, N], F32, tag="sc")
                            nc.tensor.matmul(
                                ps[:, 0:512],
                                ka[0:64, kt * P:(kt + 1) * P], qa,
                                start=True, stop=True, tile_position=(0, 0),
                            )
                            nc.tensor.matmul(
                                ps[:, 512:1024],
                                kb[64:128, kt * P:(kt + 1) * P], qb,
                                start=True, stop=True, tile_position=(64, 0),
                            )
                            es = es_pool.tile([P, N], BF16, tag="es")
                            nc.scalar.activation(es[:], ps[:], Exp, scale=float(HD) ** -0.5)
                            es_tiles.append(es)
                        if pending is not None:
                            emit_av(*pending)
                        pending = (p, qc, es_tiles)
                    emit_av(*pending)
                    emit_proj(qc)

    nc.finalize()
    return nc


_NC_CACHE = None


def _get_nc():
    global _NC_CACHE
    if _NC_CACHE is None:
        _NC_CACHE = build_nc()
    return _NC_CACHE


def prep_inputs(x, w_qkv, w_proj, b_proj):
    x = np.asarray(x, dtype=np.float32)
    w_qkv = np.asarray(w_qkv, dtype=np.float32)
    w_proj = np.asarray(w_proj, dtype=np.float32)
    b_proj = np.asarray(b_proj, dtype=np.float32)
    wqk = np.ascontiguousarray(w_qkv[:2 * C].T)          # [768, 1536]
    wv = np.ascontiguousarray(w_qkv[2 * C:].T)           # [768, 768]
    wp = np.ascontiguousarray(w_proj.T)                  # [768, 768]
    bias = np.ascontiguousarray(np.tile(b_proj[None, :], (P, 1)))  # [128, 768]
    in_maps = []
    for b in range(NCORES):
        in_maps.append({
            "xt": np.ascontiguousarray(x[b].T),          # [768, 1024]
            "wqk": wqk, "wv": wv, "wproj": wp, "bias": bias,
        })
    return in_maps


def run(in_maps, **kw):
    nc = _get_nc()
    return run_bass_kernel_spmd(nc, in_maps, list(range(NCORES)), **kw)


def kernel(x, w_qkv, w_proj, b_proj):
    res = run(prep_inputs(x, w_qkv, w_proj, b_proj))
    return np.stack([res.results[b]["out"] for b in range(NCORES)], axis=0)


# revision 8
# speedup vs baseline: 1.7130x; 1.2524x over previous
"""Fused multi-head attention block (qkv proj + attention + out proj) for
Trainium2, batch-parallel across 8 NeuronCores.

Problem shapes (hardcoded): x [8, 1024, 768], w_qkv [2304, 768],
w_proj [768, 768], b_proj [768]; H=12 heads, HD=64.

Each core processes one batch element b. Layouts:
  qkT  [2C, N]  q,k transposed (bf16): head h -> tile h//2, parts (h%2)*64..
  v_sb [N, H, 65] v natural (bf16) + ones column per head (softmax sums)
  S.T = kT.T @ qT per head, K=64 row-tiled head pairs sharing the PE array
  P.T = exp(S.T/8) on ACT (bf16, max-subtraction skipped: scores ~N(0,1),
        max ~5.5, exp < 300 so fp32 PSUM never overflows)
  [av; sums].T = [V|1].T @ P.T (bf16, M=65), normalized by broadcasting
  1/sums across partitions; attn.T (bf16) -> proj + bias.

Emission interleaves qkv/proj matmul groups into the ACT-paced attention
loop so the PE never idles (keeps HAM at K=8/8).
"""
import numpy as np

import concourse.bacc as bacc
import concourse.tile as tile
from concourse import mybir
from concourse.bass_utils import run_bass_kernel_spmd

B, N, C = 8, 1024, 768
H, HD = 12, 64
P = 128
NCORES = 8
F32 = mybir.dt.float32
F32R = mybir.dt.float32r
BF16 = mybir.dt.bfloat16
Exp = mybir.ActivationFunctionType.Exp

KC = C // P          # 6 contraction chunks of 128 over C
NT = N // P          # 8 npos tiles of 128
QC = 2               # qpos halves of 512
NPAIR = H // 2       # 6 head pairs
SCALE = float(HD) ** -0.5


def build_nc():
    nc = bacc.Bacc("TRN2", target_bir_lowering=False, debug=False)

    xt = nc.declare_dram_parameter("xt", [C, N], F32R, isOutput=False)
    wqk = nc.declare_dram_parameter("wqk", [C, 2 * C], F32R, isOutput=False)
    wv = nc.declare_dram_parameter("wv", [C, C], F32R, isOutput=False)
    wproj = nc.declare_dram_parameter("wproj", [C, C], BF16, isOutput=False)
    bias = nc.declare_dram_parameter("bias", [P, C], F32, isOutput=False)
    out = nc.declare_dram_parameter("out", [N, C], F32, isOutput=True)

    with tile.TileContext(nc) as tc:
        with tc.tile_pool(name="qk", bufs=1) as qk_pool, \
             tc.tile_pool(name="vsb", bufs=1) as v_pool, \
             tc.tile_pool(name="attnT", bufs=1) as at_pool, \
             tc.tile_pool(name="p1in", bufs=1) as p1in, \
             tc.tile_pool(name="p3in", bufs=1) as p3in, \
             tc.tile_pool(name="es", bufs=16) as es_pool, \
             tc.tile_pool(name="rr", bufs=2) as r_pool, \
             tc.tile_pool(name="osb", bufs=3) as o_pool, \
             tc.tile_pool(name="scps", bufs=2, space="PSUM") as sc_ps, \
             tc.tile_pool(name="gps", bufs=3, space="PSUM") as g_ps:

            qk_sb = [qk_pool.tile([P, N], BF16, tag=f"qk{i}", name=f"qk{i}")
                     for i in range(12)]
            v_sb = [v_pool.tile([P, H, 65], BF16, tag=f"v{i}", name=f"v{i}")
                    for i in range(NT)]
            attnT = [at_pool.tile([P, N], BF16, tag=f"at{i}", name=f"at{i}")
                     for i in range(NPAIR)]
            xt_sb = [p1in.tile([P, N], F32R, tag=f"xt{k}", name=f"xts{k}")
                     for k in range(KC)]
            wqk_sb = [p1in.tile([P, 2 * C], F32R, tag=f"wqk{k}", name=f"wqks{k}")
                      for k in range(KC)]
            wv_sb = [p1in.tile([P, C], F32R, tag=f"wv{k}", name=f"wvs{k}")
                     for k in range(KC)]
            wproj_sb = [p3in.tile([P, C], BF16, tag=f"wp{k}", name=f"wps{k}")
                        for k in range(KC)]
            bias_sb = p3in.tile([P, C], F32, tag="bias", name="biassb")

            # DMAs: xt + the wqk column slices used first (mt 0,1 / 6,7),
            # then the rest; weights for later phases last.
            for k in range(KC):
                nc.sync.dma_start(out=xt_sb[k][:], in_=xt[k * P:(k + 1) * P, :])
            for k in range(KC):
                nc.sync.dma_start(out=wqk_sb[k][:, 0:256],
                                  in_=wqk[k * P:(k + 1) * P, 0:256])
                nc.sync.dma_start(out=wqk_sb[k][:, 768:1024],
                                  in_=wqk[k * P:(k + 1) * P, 768:1024])
            for k in range(KC):
                nc.sync.dma_start(out=wv_sb[k][:], in_=wv[k * P:(k + 1) * P, :])
            for k in range(KC):
                nc.sync.dma_start(out=wqk_sb[k][:, 256:768],
                                  in_=wqk[k * P:(k + 1) * P, 256:768])
                nc.sync.dma_start(out=wqk_sb[k][:, 1024:1536],
                                  in_=wqk[k * P:(k + 1) * P, 1024:1536])
            for k in range(KC):
                nc.sync.dma_start(out=wproj_sb[k][:], in_=wproj[k * P:(k + 1) * P, :])
            nc.sync.dma_start(out=bias_sb[:], in_=bias[:, :])

            def emit_qkT(mt, nh):
                ps = g_ps.tile([P, 512], F32, tag="g", name="gq")
                for k in range(KC):
                    nc.tensor.matmul(
                        ps[:],
                        wqk_sb[k][:, mt * P:(mt + 1) * P],
                        xt_sb[k][:, nh * 512:(nh + 1) * 512],
                        start=(k == 0), stop=(k == KC - 1),
                    )
                nc.vector.tensor_copy(qk_sb[mt][:, nh * 512:(nh + 1) * 512], ps[:])

            def emit_v(nt, ci):
                c0, cw = ((0, 512), (512, 256))[ci]
                ps = g_ps.tile([P, 512], F32, tag="g", name="gv")
                for k in range(KC):
                    nc.tensor.matmul(
                        ps[:, :cw],
                        xt_sb[k][:, nt * P:(nt + 1) * P],
                        wv_sb[k][:, c0:c0 + cw],
                        start=(k == 0), stop=(k == KC - 1),
                    )
                psv = ps[:, :cw].rearrange("p (j q) -> p j q", q=64)
                nc.vector.tensor_copy(
                    v_sb[nt][:, c0 // 64:c0 // 64 + cw // 64, 0:64], psv[:])

            def emit_av(p, qc, es_tiles):
                for par in range(2):
                    h = 2 * p + par
                    av = g_ps.tile([P, 512], F32, tag="g", name="gav")
                    for kt in range(NT):
                        nc.tensor.matmul(
                            av[0:65, :],
                            v_sb[kt][:, h, :],
                            es_tiles[kt][:, par * 512:(par + 1) * 512],
                            start=(kt == 0), stop=(kt == NT - 1),
                        )
                    # evict PSUM right away so the psum slot frees without
                    # waiting on the normalization chain
                    av_sb = r_pool.tile([P, 512], F32, tag="avsb", name="avsb")
                    nc.vector.tensor_copy(av_sb[0:65, :], av[0:65, :])
                    # stock DVE op: part 64 -> part 0 (cross-quadrant ok)
                    rrow = r_pool.tile([P, 512], F32, tag="rrow", name="rrow")
                    nc.vector.tensor_copy(rrow[0:1, :], av_sb[64:65, :])
                    sbc = r_pool.tile([P, 512], F32, tag="sbc", name="sbc")
                    nc.gpsimd.partition_broadcast(sbc[0:64, :], rrow[0:1, :])
                    rbc = r_pool.tile([P, 512], F32, tag="rbc", name="rbc")
                    # custom-DVE op: base partition 0 only
                    nc.vector.reciprocal_approx_fast(rbc[0:64, :], sbc[0:64, :])
                    # 64-channel DVE op writes the head's attnT quadrant
                    nc.vector.tensor_mul(
                        attnT[p][par * 64:(par + 1) * 64, qc * 512:(qc + 1) * 512],
                        av_sb[0:64, :],
                        rbc[0:64, :])

            proj_osb = {}

            def emit_proj(nt, ci):
                c0, cw = ((0, 512), (512, 256))[ci]
                ps = g_ps.tile([P, 512], F32, tag="g", name="gp")
                for k in range(KC):
                    nc.tensor.matmul(
                        ps[:, :cw],
                        attnT[k][:, nt * P:(nt + 1) * P],
                        wproj_sb[k][:, c0:c0 + cw],
                        start=(k == 0), stop=(k == KC - 1),
                    )
                if ci == 0:
                    proj_osb[nt] = o_pool.tile([P, C], F32, tag="o", name="osb")
                o_sb = proj_osb[nt]
                nc.vector.tensor_add(o_sb[:, c0:c0 + cw], ps[:, :cw],
                                     bias_sb[:, c0:c0 + cw])
                if ci == 1:
                    nc.sync.dma_start(out=out[nt * P:(nt + 1) * P, :], in_=o_sb[:])

            def emit_scores_kt(p, qc, kt):
                ps = sc_ps.tile([P, N], F32, tag="sc", name="scps")
                nc.tensor.matmul(
                    ps[:, 0:512],
                    qk_sb[6 + p][0:64, kt * P:(kt + 1) * P],
                    qk_sb[p][0:64, qc * 512:(qc + 1) * 512],
                    start=True, stop=True, tile_position=(0, 0),
                )
                nc.tensor.matmul(
                    ps[:, 512:1024],
                    qk_sb[6 + p][64:128, kt * P:(kt + 1) * P],
                    qk_sb[p][64:128, qc * 512:(qc + 1) * 512],
                    start=True, stop=True, tile_position=(64, 0),
                )
                es = es_pool.tile([P, N], BF16, tag="es", name="es")
                nc.scalar.activation(es[:], ps[:], Exp, scale=SCALE)
                return es

            # ---------- PRE: qkT for pairs 0,1 + all of v ----------
            for mt in (0, 6, 1, 7):
                for nh in range(QC):
                    emit_qkT(mt, nh)
            for nt in range(NT):
                nc.vector.memset(v_sb[nt][:, :, 64:65], 1.0)
            for nt in range(NT):
                emit_v(nt, 0)
                emit_v(nt, 1)

            # ---------- attention with interleaved fillers ----------
            # qc0 fillers: remaining qkT M-tiles; qc1 fillers: proj of qc0 rows
            for qc in range(QC):
                pending = None
                for p in range(NPAIR):
                    if qc == 0 and p < 4:
                        fillers = [(emit_qkT, (2 + p, 0)), (emit_qkT, (2 + p, 1)),
                                   (emit_qkT, (8 + p, 0)), (emit_qkT, (8 + p, 1))]
                    elif qc == 1 and p < 4:
                        fillers = [(emit_proj, (p, 0)), (emit_proj, (p, 1))]
                    else:
                        fillers = []
                    es_tiles = []
                    for kt in range(NT):
                        es_tiles.append(emit_scores_kt(p, qc, kt))
                        if kt % 2 == 1 and fillers:
                            fn, args = fillers.pop(0)
                            fn(*args)
                    for fn, args in fillers:
                        fn(*args)
                    if pending is not None:
                        emit_av(*pending)
                    pending = (p, qc, es_tiles)
                emit_av(*pending)
            # tail: proj of qc1 rows
            for nt in range(4, NT):
                emit_proj(nt, 0)
                emit_proj(nt, 1)

    nc.finalize()
    return nc


_NC_CACHE = None


def _get_nc():
    global _NC_CACHE
    if _NC_CACHE is None:
        _NC_CACHE = build_nc()
    return _NC_CACHE


def prep_inputs(x, w_qkv, w_proj, b_proj):
    import ml_dtypes
    x = np.asarray(x, dtype=np.float32)
    w_qkv = np.asarray(w_qkv, dtype=np.float32)
    w_proj = np.asarray(w_proj, dtype=np.float32)
    b_proj = np.asarray(b_proj, dtype=np.float32)
    wqk = np.ascontiguousarray(w_qkv[:2 * C].T)          # [768, 1536]
    wv = np.ascontiguousarray(w_qkv[2 * C:].T)           # [768, 768]
    wp = np.ascontiguousarray(w_proj.T).astype(ml_dtypes.bfloat16)
    bias = np.ascontiguousarray(np.tile(b_proj[None, :], (P, 1)))  # [128, 768]
    in_maps = []
    for b in range(NCORES):
        in_maps.append({
            "xt": np.ascontiguousarray(x[b].T),          # [768, 1024]
            "wqk": wqk, "wv": wv, "wproj": wp, "bias": bias,
        })
    return in_maps


def run(in_maps, **kw):
    nc = _get_nc()
    return run_bass_kernel_spmd(nc, in_maps, list(range(NCORES)), **kw)


def kernel(x, w_qkv, w_proj, b_proj):
    res = run(prep_inputs(x, w_qkv, w_proj, b_proj))
    return np.stack([res.results[b]["out"] for b in range(NCORES)], axis=0)
